# revision 10
# baseline (speedup 1.0000x reference)
"""Block-sparse attention on 8 Trainium2 (trn2) NeuronCores via Bass/Tile.

Strategy (per spec sharding hint): shard the 32 (batch, head) units across
8 cores (4 units/core); the block layout is identical per unit so all cores
run the same SPMD program, specialized at trace time on the layout.

The layout's (block-row, block-col) entries are processed as row PAIRS
(2r, 2r+1); each pair's column multiset is chunked two columns at a time.
Per chunk the device computes:
  - sT[kpos(2 cols), q(2 rows)] = [K_ca | K_cb]^T-weights @ Q_pair^T
    (one M=128 FWL matmul; K^T combos packed contiguously on the host
    because matmul weights APs must be single-free-dim, and matmul
    operands may only live in SBUF partitions 0-63 - reading operands
    from partitions 64-127 crashes the HW),
  - E = exp(sT) on ScalarE (no max subtraction: logits are ~N(0,1) after
    the 1/sqrt(E) temperature, exp cannot overflow),
  - invalid (row, col) sub-blocks of E zeroed by GpSimd memsets; chunks
    padding an odd column count get a zeroed V-half instead,
  - out[q, 65] += E_chunk-as-weights @ [V_ca; V_cb | ones] (K=128 matmul;
    the ones column accumulates the softmax denominator for free),
accumulated per pair in PSUM banks opened/closed by K=64 zero-matmuls
(PSUM accumulate groups are bank-granular), copied to bf16 out by DVE.
Host divides by the denominator column and reassembles (B, T, H, D).
"""

import numpy as np

_CACHE = {}

NCORES = 8
BLK = 64
CH_PER_PSUM = 12   # QK chunks per PSUM tile: 12 * 128 = 1536 f32 cols = 3 banks
GROUP = 7          # row pairs per AV psum tile: 7 * 65 = 455 f32 cols = 1 bank


# --------------------------------------------------------------------------
# schedule
# --------------------------------------------------------------------------

def _build_schedule(rows, cols, nT):
    """Returns (pair_chunks, combos):
    pair_chunks[pr] = list of (combo_id, memsets) where memsets is a list of
    (col_half, row_half) E sub-blocks to zero; combos = list of
    (ca, cb_or_None) - cb None means the hi half is layout padding and the
    V/ones hi half is zeroed on the host instead."""
    from collections import Counter

    row_cols = [[] for _ in range(nT)]
    for r, c in zip(rows.tolist(), cols.tolist()):
        row_cols[int(r)].append(int(c))

    combo_ids = {}
    combos = []

    def combo(ca, cb):
        key = (ca, cb)
        if key not in combo_ids:
            combo_ids[key] = len(combos)
            combos.append(key)
        return combo_ids[key]

    pair_chunks = []
    for pr in range(nT // 2):
        m0 = Counter(row_cols[2 * pr])
        m1 = Counter(row_cols[2 * pr + 1])
        U = []
        for c in sorted(set(m0) | set(m1)):
            for j in range(max(m0.get(c, 0), m1.get(c, 0))):
                U.append((c, j < m0.get(c, 0), j < m1.get(c, 0)))
        chunks = []
        for i in range(0, len(U), 2):
            ca, a0, a1 = U[i]
            mems = []
            if not a0:
                mems.append((0, 0))
            if not a1:
                mems.append((0, 1))
            if i + 1 < len(U):
                cb, b0, b1 = U[i + 1]
                if not b0:
                    mems.append((1, 0))
                if not b1:
                    mems.append((1, 1))
                cid = combo(ca, cb)
            else:
                cid = combo(ca, None)   # padded hi half, zeroed in v2/kt2 ok
            chunks.append((cid, mems))
        pair_chunks.append(chunks)
    return pair_chunks, combos


# --------------------------------------------------------------------------
# device program
# --------------------------------------------------------------------------

def _build_program(pair_chunks, ncombo, UPC, T, nT):
    import concourse.bacc as bacc
    import concourse.mybir as mybir
    import concourse.tile as tile

    bf16 = mybir.dt.bfloat16
    f32 = mybir.dt.float32
    OUTC = (nT // 2) * 65    # out cols per unit
    NPAIR = nT // 2

    chunk_seq = []           # (pair, combo_id, memsets)
    pair_chunk_base = []
    for pr, chunks in enumerate(pair_chunks):
        pair_chunk_base.append(len(chunk_seq))
        for cid, mems in chunks:
            chunk_seq.append((pr, cid, mems))
    nchunks = len(chunk_seq)

    nc = bacc.Bacc("TRN2", target_bir_lowering=False, debug=False,
                   num_devices=NCORES)
    qt_d = nc.dram_tensor("qt", [UPC, 64, T], bf16, kind="ExternalInput")
    kt_d = nc.dram_tensor("kt2", [UPC, 64, ncombo * 128], bf16,
                          kind="ExternalInput")
    v2_d = nc.dram_tensor("v2", [UPC, 128, ncombo * 65], bf16,
                          kind="ExternalInput")
    o_d = nc.dram_tensor("o", [UPC, 128, OUTC], bf16, kind="ExternalOutput")

    with tile.TileContext(nc) as tc:
        with (
            tc.tile_pool(name="const", bufs=1) as constp,
            tc.tile_pool(name="io", bufs=2) as iop,
            tc.tile_pool(name="ep", bufs=8) as ep,
            tc.tile_pool(name="qkps", bufs=2, space="PSUM") as qkp,
            tc.tile_pool(name="avps", bufs=2, space="PSUM") as avp,
        ):
            zt = constp.tile([128, 128], bf16, tag="zeros")
            nc.gpsimd.memset(zt[:], 0.0)

            for u in range(UPC):
                qt = iop.tile([64, T], bf16, tag="qt")
                kt2 = iop.tile([64, ncombo * 128], bf16, tag="kt2")
                v2 = iop.tile([128, ncombo * 65], bf16, tag="v2")
                ot = iop.tile([128, OUTC], bf16, tag="ot")
                nc.sync.dma_start(qt[:], qt_d[u])
                nc.sync.dma_start(kt2[:], kt_d[u])
                nc.sync.dma_start(v2[:], v2_d[u])

                e_tiles = []      # batch idx -> E tile
                chunk_ref = []    # global chunk idx -> (batch idx, col off)
                emitted = [0]     # chunks emitted so far

                def emit_qk_batch():
                    base = emitted[0]
                    batch = chunk_seq[base:base + CH_PER_PSUM]
                    ncols = len(batch) * 128
                    ps = qkp.tile([128, CH_PER_PSUM * 128], f32, tag="qkps")
                    et = ep.tile([128, CH_PER_PSUM * 128], bf16, tag="e")
                    for i, (pr, cid, mems) in enumerate(batch):
                        nc.tensor.matmul(
                            ps[:, i * 128:(i + 1) * 128],
                            kt2[:, cid * 128:(cid + 1) * 128],
                            qt[:, pr * 128:(pr + 1) * 128],
                            start=True, stop=True)
                        chunk_ref.append((len(e_tiles), i * 128))
                    nc.scalar.activation(
                        et[:, 0:ncols], ps[:, 0:ncols],
                        mybir.ActivationFunctionType.Exp)
                    # zero invalid (row, col) sub-blocks on idle GpSimd
                    for i, (pr, cid, mems) in enumerate(batch):
                        for (chh, rh) in mems:
                            nc.gpsimd.memset(
                                et[chh * 64:(chh + 1) * 64,
                                   i * 128 + rh * 64:i * 128 + rh * 64 + 64],
                                0.0)
                    e_tiles.append(et)
                    emitted[0] = base + len(batch)

                for g0 in range(0, NPAIR, GROUP):
                    gpairs = list(range(g0, min(g0 + GROUP, NPAIR)))
                    # emit QK batches covering this group's chunks (+1 ahead)
                    last_chunk = pair_chunk_base[gpairs[-1]] + \
                        len(pair_chunks[gpairs[-1]])
                    while emitted[0] < min(last_chunk + CH_PER_PSUM, nchunks):
                        emit_qk_batch()

                    w = len(gpairs) * 65
                    psG = avp.tile([128, 512], f32, tag="avg")
                    # open the bank accumulation group (bank-granular):
                    # K=64 zero-matmul defines/zeroes the whole used region
                    nc.tensor.matmul(psG[:, 0:w], zt[0:64, 0:128],
                                     qt[0:64, 0:w], start=True, stop=False)
                    for gi, pr in enumerate(gpairs):
                        cb0 = pair_chunk_base[pr]
                        for ci in range(len(pair_chunks[pr])):
                            bidx, loff = chunk_ref[cb0 + ci]
                            et = e_tiles[bidx]
                            cid = pair_chunks[pr][ci][0]
                            nc.tensor.matmul(
                                psG[:, gi * 65:gi * 65 + 65],
                                et[:, loff:loff + 128],
                                v2[:, cid * 65:(cid + 1) * 65],
                                start=False, stop=False)
                    # close the group
                    nc.tensor.matmul(psG[:, 0:w], zt[0:64, 0:128],
                                     qt[0:64, 0:w], start=False, stop=True)
                    nc.vector.tensor_copy(ot[:, g0 * 65:g0 * 65 + w],
                                          psG[:, 0:w])
                nc.sync.dma_start(o_d[u], ot[:])
    nc.compile()
    return nc


# --------------------------------------------------------------------------
# host side
# --------------------------------------------------------------------------

def _prep_inputs(query, key, value, temp, combos):
    import ml_dtypes
    bf16 = ml_dtypes.bfloat16
    B, T, H, E = query.shape
    D = value.shape[-1]
    nT = T // BLK
    U = B * H
    NC2 = len(combos)
    ca_arr = np.array([c[0] for c in combos], np.int64)
    cb_arr = np.array([c[0] if c[1] is None else c[1] for c in combos],
                      np.int64)
    pad = np.array([c[1] is None for c in combos], bool)

    qt = np.ascontiguousarray(
        query.transpose(0, 2, 3, 1).reshape(U, E, T) * np.float32(temp)
    ).astype(bf16)

    # kt2[u, e, ci, 0:64] = K^T of block ca; [64:128] = block cb (or ca)
    ktt = key.transpose(0, 2, 3, 1).reshape(U, E, nT, BLK)
    kt2 = np.empty((U, E, NC2, 128), np.float32)
    kt2[:, :, :, 0:64] = ktt[:, :, ca_arr, :]
    kt2[:, :, :, 64:128] = ktt[:, :, cb_arr, :]
    kt2 = np.ascontiguousarray(kt2.reshape(U, E, NC2 * 128)).astype(bf16)

    # v2[u, 0:64, ci, :] = [V_ca | 1]; [64:128] = [V_cb | 1] (zeros if pad)
    vtt = value.transpose(0, 2, 1, 3).reshape(U, nT, BLK, D)
    v2 = np.empty((U, 128, NC2, D + 1), np.float32)
    v2[:, 0:64, :, :D] = vtt[:, ca_arr].transpose(0, 2, 1, 3)
    v2[:, 0:64, :, D] = 1.0
    v2[:, 64:128, :, :D] = vtt[:, cb_arr].transpose(0, 2, 1, 3)
    v2[:, 64:128, :, D] = 1.0
    v2[:, 64:128, pad, :] = 0.0
    v2 = np.ascontiguousarray(v2.reshape(U, 128, NC2 * (D + 1))).astype(bf16)
    return qt, kt2, v2


def _assemble(o, B, T, H, D):
    # o: [NCORES, UPC, 128, (nT/2)*65] bf16
    U = B * H
    nT = T // BLK
    o = np.asarray(o, np.float32).reshape(U, 2, BLK, nT // 2, 65)
    o = o.transpose(0, 3, 1, 2, 4).reshape(U, T, 65)  # t = (2r+ph)*64+q
    vals = o[..., :D]
    den = o[..., D:]
    out = np.where(den > 0, vals / np.where(den > 0, den, 1.0), 0.0)
    out = out.reshape(B, H, T, D).transpose(0, 2, 1, 3)
    return np.ascontiguousarray(out, np.float32)


def _get_jit_runner(nc):
    """Cached jax runner for the compiled Bass program (mirrors
    bass2jax.run_bass_via_pjrt but reuses the jitted fn across calls)."""
    import jax
    import numpy as _np
    from jax.sharding import Mesh, PartitionSpec
    from jax.experimental.shard_map import shard_map
    import concourse.mybir as mybir
    from concourse import bass2jax

    bass2jax.install_neuronx_cc_hook()

    partition_name = (nc.partition_id_tensor.name
                      if nc.partition_id_tensor else None)
    in_names, out_names, out_avals, zero_outs = [], [], [], []
    for alloc in nc.m.functions[0].allocations:
        if not isinstance(alloc, mybir.MemoryLocationSet):
            continue
        name = alloc.memorylocations[0].name
        if alloc.kind == "ExternalInput":
            if name != partition_name:
                in_names.append(name)
        elif alloc.kind == "ExternalOutput":
            shape = tuple(alloc.tensor_shape)
            dtype = mybir.dt.np(alloc.dtype)
            out_names.append(name)
            out_avals.append(jax.core.ShapedArray(shape, dtype))
            zero_outs.append(_np.zeros(shape, dtype))
    n_params = len(in_names)
    n_outs = len(out_avals)
    all_in_names = in_names + out_names
    if partition_name is not None:
        all_in_names = all_in_names + [partition_name]

    def _body(*args):
        operands = list(args)
        if partition_name is not None:
            operands.append(bass2jax.partition_id_tensor())
        outs = bass2jax._bass_exec_p.bind(
            *operands,
            out_avals=tuple(out_avals),
            in_names=tuple(all_in_names),
            out_names=tuple(out_names),
            lowering_input_output_aliases=(),
            sim_require_finite=False,
            sim_require_nnan=False,
            nc=nc,
        )
        return tuple(outs)

    devices = jax.devices()[:NCORES]
    mesh = Mesh(_np.asarray(devices), ("core",))
    in_specs = (PartitionSpec("core"),) * (n_params + n_outs)
    out_specs = (PartitionSpec("core"),) * n_outs
    donate = tuple(range(n_params, n_params + n_outs))
    sharded = jax.jit(
        shard_map(_body, mesh=mesh, in_specs=in_specs, out_specs=out_specs,
                  check_rep=False),
        donate_argnums=donate, keep_unused=True)

    def run(in_maps):
        concat_in = [
            _np.concatenate([_np.asarray(in_maps[c][n])
                             for c in range(NCORES)], axis=0)
            for n in in_names
        ]
        concat_zeros = [
            _np.zeros((NCORES * z.shape[0], *z.shape[1:]), z.dtype)
            for z in zero_outs
        ]
        out_arrs = sharded(*concat_in, *concat_zeros)
        return [
            {n: _np.asarray(out_arrs[i]).reshape(NCORES,
                                                 *out_avals[i].shape)[c]
             for i, n in enumerate(out_names)}
            for c in range(NCORES)
        ]

    return run


def _get_program(rows, cols, B, T, H, E, D):
    key_h = (rows.tobytes(), cols.tobytes(), B, T, H, E, D)
    entry = _CACHE.get(key_h)
    if entry is None:
        nT = T // BLK
        UPC = (B * H) // NCORES
        pair_chunks, combos = _build_schedule(rows, cols, nT)
        nc = _build_program(pair_chunks, len(combos), UPC, T, nT)
        runner = _get_jit_runner(nc)
        entry = (nc, runner, combos)
        _CACHE[key_h] = entry
    return entry


def _run_device(query, key, value, rows, cols, trace=False, tmpdir=None):
    B, T, H, E = query.shape
    D = value.shape[-1]
    UPC = (B * H) // NCORES
    temp = 1.0 / np.sqrt(np.float32(E))
    nc, runner, combos = _get_program(rows, cols, B, T, H, E, D)
    qt, kt2, v2 = _prep_inputs(query, key, value, temp, combos)
    in_maps = [
        {
            "qt": qt[c * UPC:(c + 1) * UPC],
            "kt2": kt2[c * UPC:(c + 1) * UPC],
            "v2": v2[c * UPC:(c + 1) * UPC],
        }
        for c in range(NCORES)
    ]
    exec_ns = None
    if trace:
        from concourse import bass_utils
        res = bass_utils.run_bass_kernel_spmd(
            nc, in_maps, list(range(NCORES)), trace=True, tmpdir=tmpdir)
        results = res.results
        exec_ns = res.exec_time_ns
    else:
        results = runner(in_maps)
    o = np.stack([results[c]["o"] for c in range(NCORES)])
    return _assemble(o, B, T, H, D), exec_ns


# --------------------------------------------------------------------------
# numpy fallback (also the small-scale oracle for dev tests)
# --------------------------------------------------------------------------

def _numpy_reference(query, key, value, rows, cols, blk):
    B, T, H, E = query.shape
    D = value.shape[-1]
    nT = T // blk
    temp = np.float32(1.0 / np.sqrt(np.float32(E)))
    q = query.transpose(0, 2, 1, 3).reshape(B, H, nT, blk, E)
    k = key.transpose(0, 2, 1, 3).reshape(B, H, nT, blk, E)
    v = value.transpose(0, 2, 1, 3).reshape(B, H, nT, blk, D)
    qb = q[:, :, rows]
    kb = k[:, :, cols]
    s = np.einsum("bhnqe,bhnke->bhnqk", qb, kb) * temp
    blk_max = s.max(axis=-1)
    row_max = np.full((nT, B, H, blk), -np.inf, np.float32)
    np.maximum.at(row_max, rows, np.moveaxis(blk_max, 2, 0))
    mx = np.moveaxis(row_max[rows], 0, 2)
    e = np.exp(s - mx[..., None])
    blk_sum = np.moveaxis(e.sum(axis=-1), 2, 0)
    row_sum = np.zeros((nT, B, H, blk), np.float32)
    np.add.at(row_sum, rows, blk_sum)
    denom = np.moveaxis(row_sum[rows], 0, 2)
    a = e / denom[..., None]
    vb = v[:, :, cols]
    ob = np.einsum("bhnqk,bhnkd->bhnqd", a, vb)
    out_rows = np.zeros((nT, B, H, blk, D), np.float32)
    np.add.at(out_rows, rows, np.moveaxis(ob, 2, 0))
    out = np.moveaxis(out_rows, 0, 2).reshape(B, H, T, D)
    return np.ascontiguousarray(out.transpose(0, 2, 1, 3))


# --------------------------------------------------------------------------
# entry point
# --------------------------------------------------------------------------

def kernel(query, key, value, layout_rows, layout_cols, block):
    query = np.asarray(query, np.float32)
    key = np.asarray(key, np.float32)
    value = np.asarray(value, np.float32)
    rows = np.asarray(layout_rows).astype(np.int64)
    cols = np.asarray(layout_cols).astype(np.int64)
    blk = int(block)

    B, T, H, E = query.shape
    D = value.shape[-1]
    try:
        assert blk == BLK and E == 64 and D == 64
        assert T % (2 * BLK) == 0 and (B * H) % NCORES == 0
        out, _ = _run_device(query, key, value, rows, cols)
        return out
    except Exception:
        import traceback
        traceback.print_exc()
        return _numpy_reference(query, key, value, rows, cols, blk)


# revision 11
# speedup vs baseline: 1.3677x; 1.3677x over previous
"""Block-sparse attention on 8 Trainium2 (trn2) NeuronCores via Bass/Tile.

Strategy (per spec sharding hint): shard the 32 (batch, head) units across
8 cores (4 units/core); the block layout is identical per unit so all cores
run the same SPMD program, specialized at trace time on the layout.

The layout's (block-row, block-col) entries are processed as row PAIRS
(2r, 2r+1); each pair's column multiset is chunked two columns at a time.
Per chunk the device computes:
  - sT[kpos(2 cols), q(2 rows)] = [K_ca | K_cb]^T-weights @ Q_pair^T
    (one M=128 FWL matmul; K^T combos packed contiguously on the host
    because matmul weights APs must be single-free-dim, and matmul
    operands may only live in SBUF partitions 0-63 - reading operands
    from partitions 64-127 crashes the HW),
  - E = exp(sT) on ScalarE (no max subtraction: logits are ~N(0,1) after
    the 1/sqrt(E) temperature, exp cannot overflow),
  - invalid (row, col) sub-blocks of E zeroed by GpSimd memsets; chunks
    padding an odd column count get a zeroed V-half instead,
  - out[q, 65] += E_chunk-as-weights @ [V_ca; V_cb | ones] (K=128 matmul;
    the ones column accumulates the softmax denominator for free),
accumulated per pair in PSUM banks opened/closed by K=64 zero-matmuls
(PSUM accumulate groups are bank-granular), copied to bf16 out by DVE.
Host divides by the denominator column and reassembles (B, T, H, D).
"""

import numpy as np

_CACHE = {}

NCORES = 8
BLK = 64
CH_PER_PSUM = 12   # QK chunks per PSUM tile: 12 * 128 = 1536 f32 cols = 3 banks
GROUP = 7          # row pairs per AV psum tile: 7 * 65 = 455 f32 cols = 1 bank


# --------------------------------------------------------------------------
# schedule
# --------------------------------------------------------------------------

def _build_schedule(rows, cols, nT):
    """Returns (pair_chunks, combos):
    pair_chunks[pr] = list of (combo_id, memsets) where memsets is a list of
    (col_half, row_half) E sub-blocks to zero; combos = list of
    (ca, cb_or_None) - cb None means the hi half is layout padding and the
    V/ones hi half is zeroed on the host instead."""
    from collections import Counter

    row_cols = [[] for _ in range(nT)]
    for r, c in zip(rows.tolist(), cols.tolist()):
        row_cols[int(r)].append(int(c))

    combo_ids = {}
    combos = []

    def combo(ca, cb):
        key = (ca, cb)
        if key not in combo_ids:
            combo_ids[key] = len(combos)
            combos.append(key)
        return combo_ids[key]

    pair_chunks = []
    for pr in range(nT // 2):
        m0 = Counter(row_cols[2 * pr])
        m1 = Counter(row_cols[2 * pr + 1])
        U = []
        for c in sorted(set(m0) | set(m1)):
            for j in range(max(m0.get(c, 0), m1.get(c, 0))):
                U.append((c, j < m0.get(c, 0), j < m1.get(c, 0)))
        chunks = []
        for i in range(0, len(U), 2):
            ca, a0, a1 = U[i]
            mems = []
            if not a0:
                mems.append((0, 0))
            if not a1:
                mems.append((0, 1))
            if i + 1 < len(U):
                cb, b0, b1 = U[i + 1]
                if not b0:
                    mems.append((1, 0))
                if not b1:
                    mems.append((1, 1))
                cid = combo(ca, cb)
            else:
                cid = combo(ca, None)   # padded hi half, zeroed in v2/kt2 ok
            chunks.append((cid, mems))
        pair_chunks.append(chunks)
    return pair_chunks, combos


# --------------------------------------------------------------------------
# device program
# --------------------------------------------------------------------------

def _build_program(pair_chunks, ncombo, UPC, T, nT):
    import concourse.bacc as bacc
    import concourse.mybir as mybir
    import concourse.tile as tile

    bf16 = mybir.dt.bfloat16
    f32 = mybir.dt.float32
    OUTC = (nT // 2) * 65    # out cols per unit
    NPAIR = nT // 2

    chunk_seq = []           # (pair, combo_id, memsets)
    pair_chunk_base = []
    for pr, chunks in enumerate(pair_chunks):
        pair_chunk_base.append(len(chunk_seq))
        for cid, mems in chunks:
            chunk_seq.append((pr, cid, mems))
    nchunks = len(chunk_seq)

    nc = bacc.Bacc("TRN2", target_bir_lowering=False, debug=False,
                   num_devices=NCORES)
    qt_d = nc.dram_tensor("qt", [UPC, 64, T], bf16, kind="ExternalInput")
    kt_d = nc.dram_tensor("kt2", [UPC, 64, ncombo * 128], bf16,
                          kind="ExternalInput")
    v2_d = nc.dram_tensor("v2", [UPC, 128, ncombo * 65], bf16,
                          kind="ExternalInput")
    o_d = nc.dram_tensor("o", [UPC, 128, OUTC], bf16, kind="ExternalOutput")

    with tile.TileContext(nc) as tc:
        with (
            tc.tile_pool(name="const", bufs=1) as constp,
            tc.tile_pool(name="io", bufs=2) as iop,
            tc.tile_pool(name="ep", bufs=8) as ep,
            tc.tile_pool(name="qkps", bufs=2, space="PSUM") as qkp,
            tc.tile_pool(name="avps", bufs=2, space="PSUM") as avp,
        ):
            zt = constp.tile([128, 128], bf16, tag="zeros")
            nc.gpsimd.memset(zt[:], 0.0)

            for u in range(UPC):
                qt = iop.tile([128, T], bf16, tag="qt")
                kt2 = iop.tile([128, ncombo * 128], bf16, tag="kt2")
                v2 = iop.tile([128, ncombo * 65], bf16, tag="v2")
                ot = iop.tile([128, OUTC], bf16, tag="ot")
                nc.sync.dma_start(qt[0:64, :], qt_d[u])
                nc.sync.dma_start(kt2[0:64, :], kt_d[u])
                nc.sync.dma_start(v2[:], v2_d[u])
                # duplicate operands into partitions 64-127: K=128 matmuls
                # engage the fast weight load path (K=64 stays LDW-bound);
                # host halves the temperature to compensate
                nc.vector.tensor_copy(qt[64:128, :], qt[0:64, :])
                nc.vector.tensor_copy(kt2[64:128, :], kt2[0:64, :])

                e_tiles = []      # batch idx -> E tile
                chunk_ref = []    # global chunk idx -> (batch idx, col off)
                emitted = [0]     # chunks emitted so far

                def emit_qk_batch():
                    base = emitted[0]
                    batch = chunk_seq[base:base + CH_PER_PSUM]
                    ncols = len(batch) * 128
                    ps = qkp.tile([128, CH_PER_PSUM * 128], f32, tag="qkps")
                    et = ep.tile([128, CH_PER_PSUM * 128], bf16, tag="e")
                    for i, (pr, cid, mems) in enumerate(batch):
                        nc.tensor.matmul(
                            ps[:, i * 128:(i + 1) * 128],
                            kt2[0:128, cid * 128:(cid + 1) * 128],
                            qt[0:128, pr * 128:(pr + 1) * 128],
                            start=True, stop=True)
                        chunk_ref.append((len(e_tiles), i * 128))
                    nc.scalar.activation(
                        et[:, 0:ncols], ps[:, 0:ncols],
                        mybir.ActivationFunctionType.Exp)
                    # zero invalid (row, col) sub-blocks on idle GpSimd
                    for i, (pr, cid, mems) in enumerate(batch):
                        for (chh, rh) in mems:
                            nc.gpsimd.memset(
                                et[chh * 64:(chh + 1) * 64,
                                   i * 128 + rh * 64:i * 128 + rh * 64 + 64],
                                0.0)
                    e_tiles.append(et)
                    emitted[0] = base + len(batch)

                for g0 in range(0, NPAIR, GROUP):
                    gpairs = list(range(g0, min(g0 + GROUP, NPAIR)))
                    # emit QK batches covering this group's chunks (+1 ahead)
                    last_chunk = pair_chunk_base[gpairs[-1]] + \
                        len(pair_chunks[gpairs[-1]])
                    while emitted[0] < min(last_chunk + CH_PER_PSUM, nchunks):
                        emit_qk_batch()

                    w = len(gpairs) * 65
                    psG = avp.tile([128, 512], f32, tag="avg")
                    # open the bank accumulation group (bank-granular):
                    # K=64 zero-matmul defines/zeroes the whole used region
                    nc.tensor.matmul(psG[:, 0:w], zt[0:64, 0:128],
                                     qt[0:64, 0:w], start=True, stop=False)
                    av_mms = []
                    for gi, pr in enumerate(gpairs):
                        cb0 = pair_chunk_base[pr]
                        for ci in range(len(pair_chunks[pr])):
                            bidx, loff = chunk_ref[cb0 + ci]
                            cid = pair_chunks[pr][ci][0]
                            av_mms.append((gi, e_tiles[bidx], loff, cid))
                    for j, (gi, et, loff, cid) in enumerate(av_mms):
                        nc.tensor.matmul(
                            psG[:, gi * 65:gi * 65 + 65],
                            et[:, loff:loff + 128],
                            v2[:, cid * 65:(cid + 1) * 65],
                            start=False, stop=(j == len(av_mms) - 1))
                    if not av_mms:
                        nc.tensor.matmul(psG[:, 0:w], zt[0:64, 0:128],
                                         qt[0:64, 0:w],
                                         start=False, stop=True)
                    nc.vector.tensor_copy(ot[:, g0 * 65:g0 * 65 + w],
                                          psG[:, 0:w])
                nc.sync.dma_start(o_d[u], ot[:])
    nc.compile()
    return nc


# --------------------------------------------------------------------------
# host side
# --------------------------------------------------------------------------

def _prep_inputs(query, key, value, temp, combos):
    import ml_dtypes
    bf16 = ml_dtypes.bfloat16
    B, T, H, E = query.shape
    D = value.shape[-1]
    nT = T // BLK
    U = B * H
    NC2 = len(combos)
    ca_arr = np.array([c[0] for c in combos], np.int64)
    cb_arr = np.array([c[0] if c[1] is None else c[1] for c in combos],
                      np.int64)
    pad = np.array([c[1] is None for c in combos], bool)

    qt = np.ascontiguousarray(
        query.transpose(0, 2, 3, 1).reshape(U, E, T) * np.float32(temp)
    ).astype(bf16)

    # kt2[u, e, ci, 0:64] = K^T of block ca; [64:128] = block cb (or ca)
    ktt = key.transpose(0, 2, 3, 1).reshape(U, E, nT, BLK)
    kt2 = np.empty((U, E, NC2, 128), np.float32)
    kt2[:, :, :, 0:64] = ktt[:, :, ca_arr, :]
    kt2[:, :, :, 64:128] = ktt[:, :, cb_arr, :]
    kt2 = np.ascontiguousarray(kt2.reshape(U, E, NC2 * 128)).astype(bf16)

    # v2[u, 0:64, ci, :] = [V_ca | 1]; [64:128] = [V_cb | 1] (zeros if pad)
    vtt = value.transpose(0, 2, 1, 3).reshape(U, nT, BLK, D)
    v2 = np.empty((U, 128, NC2, D + 1), np.float32)
    v2[:, 0:64, :, :D] = vtt[:, ca_arr].transpose(0, 2, 1, 3)
    v2[:, 0:64, :, D] = 1.0
    v2[:, 64:128, :, :D] = vtt[:, cb_arr].transpose(0, 2, 1, 3)
    v2[:, 64:128, :, D] = 1.0
    v2[:, 64:128, pad, :] = 0.0
    v2 = np.ascontiguousarray(v2.reshape(U, 128, NC2 * (D + 1))).astype(bf16)
    return qt, kt2, v2


def _assemble(o, B, T, H, D):
    # o: [NCORES, UPC, 128, (nT/2)*65] bf16
    U = B * H
    nT = T // BLK
    o = np.asarray(o, np.float32).reshape(U, 2, BLK, nT // 2, 65)
    o = o.transpose(0, 3, 1, 2, 4).reshape(U, T, 65)  # t = (2r+ph)*64+q
    vals = o[..., :D]
    den = o[..., D:]
    out = np.where(den > 0, vals / np.where(den > 0, den, 1.0), 0.0)
    out = out.reshape(B, H, T, D).transpose(0, 2, 1, 3)
    return np.ascontiguousarray(out, np.float32)


def _get_jit_runner(nc):
    """Cached jax runner for the compiled Bass program (mirrors
    bass2jax.run_bass_via_pjrt but reuses the jitted fn across calls)."""
    import jax
    import numpy as _np
    from jax.sharding import Mesh, PartitionSpec
    from jax.experimental.shard_map import shard_map
    import concourse.mybir as mybir
    from concourse import bass2jax

    bass2jax.install_neuronx_cc_hook()

    partition_name = (nc.partition_id_tensor.name
                      if nc.partition_id_tensor else None)
    in_names, out_names, out_avals, zero_outs = [], [], [], []
    for alloc in nc.m.functions[0].allocations:
        if not isinstance(alloc, mybir.MemoryLocationSet):
            continue
        name = alloc.memorylocations[0].name
        if alloc.kind == "ExternalInput":
            if name != partition_name:
                in_names.append(name)
        elif alloc.kind == "ExternalOutput":
            shape = tuple(alloc.tensor_shape)
            dtype = mybir.dt.np(alloc.dtype)
            out_names.append(name)
            out_avals.append(jax.core.ShapedArray(shape, dtype))
            zero_outs.append(_np.zeros(shape, dtype))
    n_params = len(in_names)
    n_outs = len(out_avals)
    all_in_names = in_names + out_names
    if partition_name is not None:
        all_in_names = all_in_names + [partition_name]

    def _body(*args):
        operands = list(args)
        if partition_name is not None:
            operands.append(bass2jax.partition_id_tensor())
        outs = bass2jax._bass_exec_p.bind(
            *operands,
            out_avals=tuple(out_avals),
            in_names=tuple(all_in_names),
            out_names=tuple(out_names),
            lowering_input_output_aliases=(),
            sim_require_finite=False,
            sim_require_nnan=False,
            nc=nc,
        )
        return tuple(outs)

    devices = jax.devices()[:NCORES]
    mesh = Mesh(_np.asarray(devices), ("core",))
    in_specs = (PartitionSpec("core"),) * (n_params + n_outs)
    out_specs = (PartitionSpec("core"),) * n_outs
    donate = tuple(range(n_params, n_params + n_outs))
    sharded = jax.jit(
        shard_map(_body, mesh=mesh, in_specs=in_specs, out_specs=out_specs,
                  check_rep=False),
        donate_argnums=donate, keep_unused=True)

    def run(in_maps):
        concat_in = [
            _np.concatenate([_np.asarray(in_maps[c][n])
                             for c in range(NCORES)], axis=0)
            for n in in_names
        ]
        concat_zeros = [
            _np.zeros((NCORES * z.shape[0], *z.shape[1:]), z.dtype)
            for z in zero_outs
        ]
        out_arrs = sharded(*concat_in, *concat_zeros)
        return [
            {n: _np.asarray(out_arrs[i]).reshape(NCORES,
                                                 *out_avals[i].shape)[c]
             for i, n in enumerate(out_names)}
            for c in range(NCORES)
        ]

    return run


def _get_program(rows, cols, B, T, H, E, D):
    key_h = (rows.tobytes(), cols.tobytes(), B, T, H, E, D)
    entry = _CACHE.get(key_h)
    if entry is None:
        nT = T // BLK
        UPC = (B * H) // NCORES
        pair_chunks, combos = _build_schedule(rows, cols, nT)
        nc = _build_program(pair_chunks, len(combos), UPC, T, nT)
        runner = _get_jit_runner(nc)
        entry = (nc, runner, combos)
        _CACHE[key_h] = entry
    return entry


def _run_device(query, key, value, rows, cols, trace=False, tmpdir=None):
    B, T, H, E = query.shape
    D = value.shape[-1]
    UPC = (B * H) // NCORES
    temp = 0.5 / np.sqrt(np.float32(E))   # halved: K padded 64->128 by dup
    nc, runner, combos = _get_program(rows, cols, B, T, H, E, D)
    qt, kt2, v2 = _prep_inputs(query, key, value, temp, combos)
    in_maps = [
        {
            "qt": qt[c * UPC:(c + 1) * UPC],
            "kt2": kt2[c * UPC:(c + 1) * UPC],
            "v2": v2[c * UPC:(c + 1) * UPC],
        }
        for c in range(NCORES)
    ]
    exec_ns = None
    if trace:
        from concourse import bass_utils
        res = bass_utils.run_bass_kernel_spmd(
            nc, in_maps, list(range(NCORES)), trace=True, tmpdir=tmpdir)
        results = res.results
        exec_ns = res.exec_time_ns
    else:
        results = runner(in_maps)
    o = np.stack([results[c]["o"] for c in range(NCORES)])
    return _assemble(o, B, T, H, D), exec_ns


# --------------------------------------------------------------------------
# numpy fallback (also the small-scale oracle for dev tests)
# --------------------------------------------------------------------------

def _numpy_reference(query, key, value, rows, cols, blk):
    B, T, H, E = query.shape
    D = value.shape[-1]
    nT = T // blk
    temp = np.float32(1.0 / np.sqrt(np.float32(E)))
    q = query.transpose(0, 2, 1, 3).reshape(B, H, nT, blk, E)
    k = key.transpose(0, 2, 1, 3).reshape(B, H, nT, blk, E)
    v = value.transpose(0, 2, 1, 3).reshape(B, H, nT, blk, D)
    qb = q[:, :, rows]
    kb = k[:, :, cols]
    s = np.einsum("bhnqe,bhnke->bhnqk", qb, kb) * temp
    blk_max = s.max(axis=-1)
    row_max = np.full((nT, B, H, blk), -np.inf, np.float32)
    np.maximum.at(row_max, rows, np.moveaxis(blk_max, 2, 0))
    mx = np.moveaxis(row_max[rows], 0, 2)
    e = np.exp(s - mx[..., None])
    blk_sum = np.moveaxis(e.sum(axis=-1), 2, 0)
    row_sum = np.zeros((nT, B, H, blk), np.float32)
    np.add.at(row_sum, rows, blk_sum)
    denom = np.moveaxis(row_sum[rows], 0, 2)
    a = e / denom[..., None]
    vb = v[:, :, cols]
    ob = np.einsum("bhnqk,bhnkd->bhnqd", a, vb)
    out_rows = np.zeros((nT, B, H, blk, D), np.float32)
    np.add.at(out_rows, rows, np.moveaxis(ob, 2, 0))
    out = np.moveaxis(out_rows, 0, 2).reshape(B, H, T, D)
    return np.ascontiguousarray(out.transpose(0, 2, 1, 3))


# --------------------------------------------------------------------------
# entry point
# --------------------------------------------------------------------------

def kernel(query, key, value, layout_rows, layout_cols, block):
    query = np.asarray(query, np.float32)
    key = np.asarray(key, np.float32)
    value = np.asarray(value, np.float32)
    rows = np.asarray(layout_rows).astype(np.int64)
    cols = np.asarray(layout_cols).astype(np.int64)
    blk = int(block)

    B, T, H, E = query.shape
    D = value.shape[-1]
    try:
        assert blk == BLK and E == 64 and D == 64
        assert T % (2 * BLK) == 0 and (B * H) % NCORES == 0
        out, _ = _run_device(query, key, value, rows, cols)
        return out
    except Exception:
        import traceback
        traceback.print_exc()
        return _numpy_reference(query, key, value, rows, cols, blk)


# revision 13
# speedup vs baseline: 1.4775x; 1.0802x over previous
"""Block-sparse attention on 8 Trainium2 (trn2) NeuronCores via Bass/Tile.

Strategy (per spec sharding hint): shard the 32 (batch, head) units across
8 cores (4 units/core); the block layout is identical per unit so all cores
run the same SPMD program, specialized at trace time on the layout.

The layout's (block-row, block-col) entries are processed as row PAIRS
(2r, 2r+1); each pair's column multiset is chunked two columns at a time.
Per chunk the device computes:
  - sT[kpos(2 cols), q(2 rows)] = [K_ca | K_cb]^T-weights @ Q_pair^T
    (one M=128 FWL matmul; K^T combos packed contiguously on the host
    because matmul weights APs must be single-free-dim, and matmul
    operands may only live in SBUF partitions 0-63 - reading operands
    from partitions 64-127 crashes the HW),
  - E = exp(sT) on ScalarE (no max subtraction: logits are ~N(0,1) after
    the 1/sqrt(E) temperature, exp cannot overflow),
  - invalid (row, col) sub-blocks of E zeroed by GpSimd memsets; chunks
    padding an odd column count get a zeroed V-half instead,
  - out[q, 65] += E_chunk-as-weights @ [V_ca; V_cb | ones] (K=128 matmul;
    the ones column accumulates the softmax denominator for free),
accumulated per pair in PSUM banks opened/closed by K=64 zero-matmuls
(PSUM accumulate groups are bank-granular), copied to bf16 out by DVE.
Host divides by the denominator column and reassembles (B, T, H, D).
"""

import numpy as np

_CACHE = {}

NCORES = 8
BLK = 64
CH_PER_PSUM = 12   # QK chunks per PSUM tile: 12 * 128 = 1536 f32 cols = 3 banks
GROUP = 7          # row pairs per AV psum tile: 7 * 65 = 455 f32 cols = 1 bank


# --------------------------------------------------------------------------
# schedule
# --------------------------------------------------------------------------

def _build_schedule(rows, cols, nT):
    """Returns (pair_chunks, combos):
    pair_chunks[pr] = list of (combo_id, memsets) where memsets is a list of
    (col_half, row_half) E sub-blocks to zero; combos = list of
    (ca, cb_or_None) - cb None means the hi half is layout padding and the
    V/ones hi half is zeroed on the host instead."""
    from collections import Counter

    row_cols = [[] for _ in range(nT)]
    for r, c in zip(rows.tolist(), cols.tolist()):
        row_cols[int(r)].append(int(c))

    combo_ids = {}
    combos = []

    def combo(ca, cb):
        key = (ca, cb)
        if key not in combo_ids:
            combo_ids[key] = len(combos)
            combos.append(key)
        return combo_ids[key]

    pair_chunks = []
    for pr in range(nT // 2):
        m0 = Counter(row_cols[2 * pr])
        m1 = Counter(row_cols[2 * pr + 1])
        U = []
        for c in sorted(set(m0) | set(m1)):
            for j in range(max(m0.get(c, 0), m1.get(c, 0))):
                U.append((c, j < m0.get(c, 0), j < m1.get(c, 0)))
        chunks = []
        for i in range(0, len(U), 2):
            ca, a0, a1 = U[i]
            mems = []
            if not a0:
                mems.append((0, 0))
            if not a1:
                mems.append((0, 1))
            if i + 1 < len(U):
                cb, b0, b1 = U[i + 1]
                if not b0:
                    mems.append((1, 0))
                if not b1:
                    mems.append((1, 1))
                cid = combo(ca, cb)
            else:
                cid = combo(ca, None)   # padded hi half, zeroed in v2/kt2 ok
            chunks.append((cid, mems))
        pair_chunks.append(chunks)
    return pair_chunks, combos


# --------------------------------------------------------------------------
# device program
# --------------------------------------------------------------------------

def _build_program(pair_chunks, ncombo, UPC, T, nT):
    import concourse.bacc as bacc
    import concourse.mybir as mybir
    import concourse.tile as tile

    bf16 = mybir.dt.bfloat16
    f32 = mybir.dt.float32
    OUTC = (nT // 2) * 65    # out cols per unit
    NPAIR = nT // 2

    chunk_seq = []           # (pair, combo_id, memsets)
    pair_chunk_base = []
    for pr, chunks in enumerate(pair_chunks):
        pair_chunk_base.append(len(chunk_seq))
        for cid, mems in chunks:
            chunk_seq.append((pr, cid, mems))
    nchunks = len(chunk_seq)

    nc = bacc.Bacc("TRN2", target_bir_lowering=False, debug=False,
                   num_devices=NCORES)
    qt_d = nc.dram_tensor("qt", [UPC, 64, T], bf16, kind="ExternalInput")
    kt_d = nc.dram_tensor("kt2", [UPC, 64, ncombo * 128], bf16,
                          kind="ExternalInput")
    v2_d = nc.dram_tensor("v2", [UPC, 128, ncombo * 65], bf16,
                          kind="ExternalInput")
    o_d = nc.dram_tensor("o", [UPC, 128, OUTC], bf16, kind="ExternalOutput")

    KW = ncombo * 128
    VW = ncombo * 65

    with tile.TileContext(nc) as tc:
        with (
            tc.tile_pool(name="const", bufs=1) as constp,
            tc.tile_pool(name="io", bufs=2) as iop,
            tc.tile_pool(name="ep", bufs=8) as ep,
            tc.tile_pool(name="qkps", bufs=2, space="PSUM") as qkp,
            tc.tile_pool(name="avps", bufs=2, space="PSUM") as avp,
        ):
            zt = constp.tile([128, 128], bf16, tag="zeros")
            nc.gpsimd.memset(zt[:], 0.0)

            def load(u):
                """DMA + K-dup for unit u. Split into pieces so QK matmuls
                can start on the first piece (Tile deps are region
                granular); combo ids are first-use ordered."""
                qt = iop.tile([128, T], bf16, tag="qt")
                kt2 = iop.tile([128, KW], bf16, tag="kt2")
                v2 = iop.tile([128, VW], bf16, tag="v2")
                ot = iop.tile([128, OUTC], bf16, tag="ot")
                nc.sync.dma_start(qt[0:64, :], qt_d[u])
                nc.vector.tensor_copy(qt[64:128, :], qt[0:64, :])
                kh = (KW // 3) // 128 * 128
                for a, b in ((0, kh), (kh, 2 * kh), (2 * kh, KW)):
                    nc.sync.dma_start(kt2[0:64, a:b], kt_d[u][:, a:b])
                    nc.vector.tensor_copy(kt2[64:128, a:b], kt2[0:64, a:b])
                vh = (VW // 2) // 65 * 65
                for a, b in ((0, vh), (vh, VW)):
                    nc.sync.dma_start(v2[:, a:b], v2_d[u][:, a:b])
                return dict(qt=qt, kt2=kt2, v2=v2, ot=ot)

            cur = load(0)
            for u in range(UPC):
                qt, kt2, v2, ot = cur["qt"], cur["kt2"], cur["v2"], cur["ot"]
                e_tiles = []      # batch idx -> E tile
                chunk_ref = []    # global chunk idx -> (batch idx, col off)
                emitted = [0]     # chunks emitted so far

                def emit_qk_batch():
                    base = emitted[0]
                    batch = chunk_seq[base:base + CH_PER_PSUM]
                    ncols = len(batch) * 128
                    ps = qkp.tile([128, CH_PER_PSUM * 128], f32, tag="qkps")
                    et = ep.tile([128, CH_PER_PSUM * 128], bf16, tag="e")
                    for i, (pr, cid, mems) in enumerate(batch):
                        nc.tensor.matmul(
                            ps[:, i * 128:(i + 1) * 128],
                            kt2[0:128, cid * 128:(cid + 1) * 128],
                            qt[0:128, pr * 128:(pr + 1) * 128],
                            start=True, stop=True)
                        chunk_ref.append((len(e_tiles), i * 128))
                    nc.scalar.activation(
                        et[:, 0:ncols], ps[:, 0:ncols],
                        mybir.ActivationFunctionType.Exp)
                    # zero invalid (row, col) sub-blocks on idle GpSimd
                    for i, (pr, cid, mems) in enumerate(batch):
                        for (chh, rh) in mems:
                            nc.gpsimd.memset(
                                et[chh * 64:(chh + 1) * 64,
                                   i * 128 + rh * 64:i * 128 + rh * 64 + 64],
                                0.0)
                    e_tiles.append(et)
                    emitted[0] = base + len(batch)

                nxt = None
                for gidx, g0 in enumerate(range(0, NPAIR, GROUP)):
                    gpairs = list(range(g0, min(g0 + GROUP, NPAIR)))
                    # emit QK batches covering this group's chunks (+1 ahead)
                    last_chunk = pair_chunk_base[gpairs[-1]] + \
                        len(pair_chunks[gpairs[-1]])
                    while emitted[0] < min(last_chunk + CH_PER_PSUM, nchunks):
                        emit_qk_batch()
                    if gidx == 1 and u + 1 < UPC:
                        # prefetch next unit's inputs mid-unit so the next
                        # unit's QK can start immediately after this one
                        nxt = load(u + 1)


                    w = len(gpairs) * 65
                    psG = avp.tile([128, 512], f32, tag="avg")
                    # open the bank accumulation group (bank-granular):
                    # K=64 zero-matmul defines/zeroes the whole used region
                    nc.tensor.matmul(psG[:, 0:w], zt[0:64, 0:128],
                                     qt[0:64, 0:w], start=True, stop=False)
                    av_mms = []
                    for gi, pr in enumerate(gpairs):
                        cb0 = pair_chunk_base[pr]
                        for ci in range(len(pair_chunks[pr])):
                            bidx, loff = chunk_ref[cb0 + ci]
                            cid = pair_chunks[pr][ci][0]
                            av_mms.append((gi, e_tiles[bidx], loff, cid))
                    for j, (gi, et, loff, cid) in enumerate(av_mms):
                        nc.tensor.matmul(
                            psG[:, gi * 65:gi * 65 + 65],
                            et[:, loff:loff + 128],
                            v2[:, cid * 65:(cid + 1) * 65],
                            start=False, stop=(j == len(av_mms) - 1))
                    if not av_mms:
                        nc.tensor.matmul(psG[:, 0:w], zt[0:64, 0:128],
                                         qt[0:64, 0:w],
                                         start=False, stop=True)
                    nc.vector.tensor_copy(ot[:, g0 * 65:g0 * 65 + w],
                                          psG[:, 0:w])
                nc.sync.dma_start(o_d[u], ot[:])
                if u + 1 < UPC:
                    if nxt is None:
                        nxt = load(u + 1)
                    cur = nxt
    nc.compile()
    return nc


# --------------------------------------------------------------------------
# host side
# --------------------------------------------------------------------------

def _prep_inputs(query, key, value, temp, combos):
    import ml_dtypes
    bf16 = ml_dtypes.bfloat16
    B, T, H, E = query.shape
    D = value.shape[-1]
    nT = T // BLK
    U = B * H
    NC2 = len(combos)
    ca_arr = np.array([c[0] for c in combos], np.int64)
    cb_arr = np.array([c[0] if c[1] is None else c[1] for c in combos],
                      np.int64)
    pad = np.array([c[1] is None for c in combos], bool)

    qt = np.ascontiguousarray(
        query.transpose(0, 2, 3, 1).reshape(U, E, T) * np.float32(temp)
    ).astype(bf16)

    # kt2[u, e, ci, 0:64] = K^T of block ca; [64:128] = block cb (or ca)
    ktt = key.transpose(0, 2, 3, 1).reshape(U, E, nT, BLK)
    kt2 = np.empty((U, E, NC2, 128), np.float32)
    kt2[:, :, :, 0:64] = ktt[:, :, ca_arr, :]
    kt2[:, :, :, 64:128] = ktt[:, :, cb_arr, :]
    kt2 = np.ascontiguousarray(kt2.reshape(U, E, NC2 * 128)).astype(bf16)

    # v2[u, 0:64, ci, :] = [V_ca | 1]; [64:128] = [V_cb | 1] (zeros if pad)
    vtt = value.transpose(0, 2, 1, 3).reshape(U, nT, BLK, D)
    v2 = np.empty((U, 128, NC2, D + 1), np.float32)
    v2[:, 0:64, :, :D] = vtt[:, ca_arr].transpose(0, 2, 1, 3)
    v2[:, 0:64, :, D] = 1.0
    v2[:, 64:128, :, :D] = vtt[:, cb_arr].transpose(0, 2, 1, 3)
    v2[:, 64:128, :, D] = 1.0
    v2[:, 64:128, pad, :] = 0.0
    v2 = np.ascontiguousarray(v2.reshape(U, 128, NC2 * (D + 1))).astype(bf16)
    return qt, kt2, v2


def _assemble(o, B, T, H, D):
    # o: [NCORES, UPC, 128, (nT/2)*65] bf16
    U = B * H
    nT = T // BLK
    o = np.asarray(o, np.float32).reshape(U, 2, BLK, nT // 2, 65)
    o = o.transpose(0, 3, 1, 2, 4).reshape(U, T, 65)  # t = (2r+ph)*64+q
    vals = o[..., :D]
    den = o[..., D:]
    out = np.where(den > 0, vals / np.where(den > 0, den, 1.0), 0.0)
    out = out.reshape(B, H, T, D).transpose(0, 2, 1, 3)
    return np.ascontiguousarray(out, np.float32)


def _get_jit_runner(nc):
    """Cached jax runner for the compiled Bass program (mirrors
    bass2jax.run_bass_via_pjrt but reuses the jitted fn across calls)."""
    import jax
    import numpy as _np
    from jax.sharding import Mesh, PartitionSpec
    from jax.experimental.shard_map import shard_map
    import concourse.mybir as mybir
    from concourse import bass2jax

    bass2jax.install_neuronx_cc_hook()

    partition_name = (nc.partition_id_tensor.name
                      if nc.partition_id_tensor else None)
    in_names, out_names, out_avals, zero_outs = [], [], [], []
    for alloc in nc.m.functions[0].allocations:
        if not isinstance(alloc, mybir.MemoryLocationSet):
            continue
        name = alloc.memorylocations[0].name
        if alloc.kind == "ExternalInput":
            if name != partition_name:
                in_names.append(name)
        elif alloc.kind == "ExternalOutput":
            shape = tuple(alloc.tensor_shape)
            dtype = mybir.dt.np(alloc.dtype)
            out_names.append(name)
            out_avals.append(jax.core.ShapedArray(shape, dtype))
            zero_outs.append(_np.zeros(shape, dtype))
    n_params = len(in_names)
    n_outs = len(out_avals)
    all_in_names = in_names + out_names
    if partition_name is not None:
        all_in_names = all_in_names + [partition_name]

    def _body(*args):
        operands = list(args)
        if partition_name is not None:
            operands.append(bass2jax.partition_id_tensor())
        outs = bass2jax._bass_exec_p.bind(
            *operands,
            out_avals=tuple(out_avals),
            in_names=tuple(all_in_names),
            out_names=tuple(out_names),
            lowering_input_output_aliases=(),
            sim_require_finite=False,
            sim_require_nnan=False,
            nc=nc,
        )
        return tuple(outs)

    devices = jax.devices()[:NCORES]
    mesh = Mesh(_np.asarray(devices), ("core",))
    in_specs = (PartitionSpec("core"),) * (n_params + n_outs)
    out_specs = (PartitionSpec("core"),) * n_outs
    donate = tuple(range(n_params, n_params + n_outs))
    sharded = jax.jit(
        shard_map(_body, mesh=mesh, in_specs=in_specs, out_specs=out_specs,
                  check_rep=False),
        donate_argnums=donate, keep_unused=True)

    def run(in_maps):
        concat_in = [
            _np.concatenate([_np.asarray(in_maps[c][n])
                             for c in range(NCORES)], axis=0)
            for n in in_names
        ]
        concat_zeros = [
            _np.zeros((NCORES * z.shape[0], *z.shape[1:]), z.dtype)
            for z in zero_outs
        ]
        out_arrs = sharded(*concat_in, *concat_zeros)
        return [
            {n: _np.asarray(out_arrs[i]).reshape(NCORES,
                                                 *out_avals[i].shape)[c]
             for i, n in enumerate(out_names)}
            for c in range(NCORES)
        ]

    return run


def _get_program(rows, cols, B, T, H, E, D):
    key_h = (rows.tobytes(), cols.tobytes(), B, T, H, E, D)
    entry = _CACHE.get(key_h)
    if entry is None:
        nT = T // BLK
        UPC = (B * H) // NCORES
        pair_chunks, combos = _build_schedule(rows, cols, nT)
        nc = _build_program(pair_chunks, len(combos), UPC, T, nT)
        runner = _get_jit_runner(nc)
        entry = (nc, runner, combos)
        _CACHE[key_h] = entry
    return entry


def _run_device(query, key, value, rows, cols, trace=False, tmpdir=None):
    B, T, H, E = query.shape
    D = value.shape[-1]
    UPC = (B * H) // NCORES
    temp = 0.5 / np.sqrt(np.float32(E))   # halved: K padded 64->128 by dup
    nc, runner, combos = _get_program(rows, cols, B, T, H, E, D)
    qt, kt2, v2 = _prep_inputs(query, key, value, temp, combos)
    in_maps = [
        {
            "qt": qt[c * UPC:(c + 1) * UPC],
            "kt2": kt2[c * UPC:(c + 1) * UPC],
            "v2": v2[c * UPC:(c + 1) * UPC],
        }
        for c in range(NCORES)
    ]
    exec_ns = None
    if trace:
        from concourse import bass_utils
        res = bass_utils.run_bass_kernel_spmd(
            nc, in_maps, list(range(NCORES)), trace=True, tmpdir=tmpdir)
        results = res.results
        exec_ns = res.exec_time_ns
    else:
        results = runner(in_maps)
    o = np.stack([results[c]["o"] for c in range(NCORES)])
    return _assemble(o, B, T, H, D), exec_ns


# --------------------------------------------------------------------------
# numpy fallback (also the small-scale oracle for dev tests)
# --------------------------------------------------------------------------

def _numpy_reference(query, key, value, rows, cols, blk):
    B, T, H, E = query.shape
    D = value.shape[-1]
    nT = T // blk
    temp = np.float32(1.0 / np.sqrt(np.float32(E)))
    q = query.transpose(0, 2, 1, 3).reshape(B, H, nT, blk, E)
    k = key.transpose(0, 2, 1, 3).reshape(B, H, nT, blk, E)
    v = value.transpose(0, 2, 1, 3).reshape(B, H, nT, blk, D)
    qb = q[:, :, rows]
    kb = k[:, :, cols]
    s = np.einsum("bhnqe,bhnke->bhnqk", qb, kb) * temp
    blk_max = s.max(axis=-1)
    row_max = np.full((nT, B, H, blk), -np.inf, np.float32)
    np.maximum.at(row_max, rows, np.moveaxis(blk_max, 2, 0))
    mx = np.moveaxis(row_max[rows], 0, 2)
    e = np.exp(s - mx[..., None])
    blk_sum = np.moveaxis(e.sum(axis=-1), 2, 0)
    row_sum = np.zeros((nT, B, H, blk), np.float32)
    np.add.at(row_sum, rows, blk_sum)
    denom = np.moveaxis(row_sum[rows], 0, 2)
    a = e / denom[..., None]
    vb = v[:, :, cols]
    ob = np.einsum("bhnqk,bhnkd->bhnqd", a, vb)
    out_rows = np.zeros((nT, B, H, blk, D), np.float32)
    np.add.at(out_rows, rows, np.moveaxis(ob, 2, 0))
    out = np.moveaxis(out_rows, 0, 2).reshape(B, H, T, D)
    return np.ascontiguousarray(out.transpose(0, 2, 1, 3))


# --------------------------------------------------------------------------
# entry point
# --------------------------------------------------------------------------

def kernel(query, key, value, layout_rows, layout_cols, block):
    query = np.asarray(query, np.float32)
    key = np.asarray(key, np.float32)
    value = np.asarray(value, np.float32)
    rows = np.asarray(layout_rows).astype(np.int64)
    cols = np.asarray(layout_cols).astype(np.int64)
    blk = int(block)

    B, T, H, E = query.shape
    D = value.shape[-1]
    try:
        assert blk == BLK and E == 64 and D == 64
        assert T % (2 * BLK) == 0 and (B * H) % NCORES == 0
        out, _ = _run_device(query, key, value, rows, cols)
        return out
    except Exception:
        import traceback
        traceback.print_exc()
        return _numpy_reference(query, key, value, rows, cols, blk)
